# revision 11
# baseline (speedup 1.0000x reference)
"""GATv2 block (N=50000, F=128, H=4, C=32, E=800000) on 8 Trainium2 NeuronCores.

Strategy (dst-node sharding, degree-rank balanced):
  - Nodes assigned to cores by degree rank (rank r -> core r % 8): each core
    owns 6250 destinations and every core's batch b has the same max degree
    -> one shared SPMD program, balanced edge counts.
  - Per core, destinations are processed in 49 batches of 128 nodes
    (partition = node). Slot 0 of every node is its self-loop (computed by a
    PE matmul, not gathered); slots 1..D-1 are neighbor edges whose xl rows
    are fetched with one batched SWDGE dma_gather per batch (int16 indices,
    table base at row 32768 so signed indices span all 50000 rows).
  - Softmax + aggregation are free-dim vector ops (partition = dst node):
      z = xl[src] + xr[dst]; m = prelu(z, 0.2); lg[d,h] = sum_c m*att
      ex = exp(lg) * mask; den = sum_d ex; alpha = ex/den
      out = relu(sum_d alpha * xl[src] + bias)
  - xl = x @ Wl is computed on-device per core into a DRAM table for the
    gathers; xr and xl_self come from batch-ordered x (xpT input).
"""
import sys

sys.path.insert(0, "/opt/trn_rl_repo")

import numpy as np

N, F, H, C = 50000, 128, 4, 32
NC, P, HALF = 8, 128, 32768
NEG_SLOPE = 0.2
NLOC = N // NC            # 6250
NB = (NLOC + P - 1) // P  # 49
NLOC_PAD = NB * P         # 6272
XT_TILE = 2048


def _host_prep(src_t, dst_t):
    """Batch structure with self-loop at slot 0 of each node."""
    deg = np.bincount(dst_t, minlength=N)          # includes self loop
    rank = np.argsort(deg, kind="stable")
    node_core = np.empty(N, dtype=np.int64)
    node_slot = np.empty(N, dtype=np.int64)
    node_core[rank] = np.arange(N) % NC
    node_slot[rank] = np.arange(N) // NC

    eorder = np.argsort(dst_t, kind="stable")
    starts = np.zeros(N + 1, dtype=np.int64)
    starts[1:] = np.cumsum(deg)

    node_batch = node_slot // P
    D_b = np.zeros(NB, dtype=np.int64)
    np.maximum.at(D_b, node_batch, deg)
    D_b = np.maximum(D_b, 2)                       # >=1 gathered slot
    a_off = np.concatenate([[0], np.cumsum(D_b)])[:-1]
    ADW = int(D_b.sum())
    GW = int((D_b - 1).sum())                      # gathered slots per node
    g_off = np.concatenate([[0], np.cumsum(D_b - 1)])[:-1]
    IW = 8 * GW

    Et = src_t.shape[0]
    idx_pack = np.zeros((NC, P, IW), dtype=np.int16)
    msk_pack = np.zeros((NC, P, ADW), dtype=np.float32)
    nodes_cb = np.full((NC, NLOC_PAD), -1, dtype=np.int64)
    e_part = np.zeros(Et, dtype=np.int64)
    e_d = np.zeros(Et, dtype=np.int64)
    e_b = node_batch[dst_t]
    e_core = node_core[dst_t]
    selfloop_eid = Et - N + np.arange(N)           # self loop of node n

    for c in range(NC):
        has = node_core == c
        slot_nodes = np.full(NLOC_PAD, -1, dtype=np.int64)
        slot_nodes[node_slot[has]] = np.nonzero(has)[0]
        for b in range(NB):
            D = int(D_b[b])
            nb = slot_nodes[b * P:(b + 1) * P].copy()
            degs = np.where(nb >= 0, deg[np.maximum(nb, 0)], 0)
            # partition-127 node must have a tail-safe last gather slot:
            # pad (deg < D) or a hi (>= HALF) non-self edge to place last.
            reorder_hi_last = False
            if degs[P - 1] >= D:
                cand = np.nonzero(degs < D)[0]
                if len(cand):
                    j = int(cand[0])
                    nb[[j, P - 1]] = nb[[P - 1, j]]
                else:
                    ok = -1
                    for j in range(P - 1, -1, -1):
                        n = nb[j]
                        if n < 0:
                            continue
                        ee = eorder[starts[n]:starts[n + 1]]
                        if (src_t[ee[ee != selfloop_eid[n]]] >= HALF).any():
                            ok = j
                            break
                    assert ok >= 0, "no hi-src edge in batch"
                    nb[[ok, P - 1]] = nb[[P - 1, ok]]
                    reorder_hi_last = True
            nodes_cb[c, b * P:(b + 1) * P] = nb

            gi = np.full((P, D - 1), HALF, dtype=np.int64)  # pads: row HALF
            for j in range(P):
                n = nb[j]
                if n < 0:
                    continue
                ee = eorder[starts[n]:starts[n + 1]]
                se = selfloop_eid[n]
                ee_n = ee[ee != se]                  # non-self edges
                if j == P - 1 and reorder_hi_last:
                    ee_n = ee_n[np.argsort(src_t[ee_n] >= HALF, kind="stable")]
                # slot 0 = self loop
                e_part[se] = j
                e_d[se] = 0
                msk_pack[c, j, a_off[b]] = 1.0
                k = len(ee_n)
                gi[j, :k] = src_t[ee_n]
                e_part[ee_n] = j
                e_d[ee_n] = 1 + np.arange(k)
                msk_pack[c, j, a_off[b] + 1:a_off[b] + 1 + k] = 1.0
            assert gi[P - 1, D - 2] >= HALF, (c, b)
            lst = gi.T.reshape(P * (D - 1))
            w = (lst - HALF).astype(np.int16).reshape(-1, 16).T
            idx_pack[c, :, 8 * g_off[b]:8 * (g_off[b] + D - 1)] = np.tile(w, (8, 1))

    return dict(D_b=D_b, a_off=a_off, g_off=g_off, ADW=ADW, GW=GW, IW=IW,
                idx_pack=idx_pack, msk_pack=msk_pack, nodes_cb=nodes_cb,
                e_core=e_core, e_part=e_part, e_d=e_d, e_b=e_b)


def _build_program(D_b, a_off, g_off, ADW, GW, IW):
    import concourse.bacc as bacc
    import concourse.mybir as mybir
    import concourse.tile as tile

    f32 = mybir.dt.float32
    Alu = mybir.AluOpType
    Act = mybir.ActivationFunctionType

    nc = bacc.Bacc("TRN2", target_bir_lowering=False)

    t_xT = nc.dram_tensor("xT", [F, N], f32, kind="ExternalInput")
    t_xpT = nc.dram_tensor("xpT", [F, NLOC_PAD], f32, kind="ExternalInput")
    t_Wl = nc.dram_tensor("Wl", [F, F], f32, kind="ExternalInput")
    t_Wr = nc.dram_tensor("Wr", [F, F], f32, kind="ExternalInput")
    t_att = nc.dram_tensor("attr", [P, F], f32, kind="ExternalInput")
    t_bias = nc.dram_tensor("biasr", [P, F], f32, kind="ExternalInput")
    t_idx = nc.dram_tensor("idxp", [P, IW], mybir.dt.int16, kind="ExternalInput")
    t_msk = nc.dram_tensor("mskp", [P, ADW], f32, kind="ExternalInput")
    t_out = nc.dram_tensor("outb", [P, NB * F], f32, kind="ExternalOutput")
    t_alpha = nc.dram_tensor("alphab", [P, 4 * ADW], f32, kind="ExternalOutput")
    t_xl = nc.dram_tensor("xl_table", [N, F], f32)

    DMAX = int(max(D_b))
    n_xtile = (N + XT_TILE - 1) // XT_TILE

    with tile.TileContext(nc) as tc:
        with tc.tile_pool(name="const", bufs=1) as cp, \
             tc.tile_pool(name="ph1", bufs=2) as p1, \
             tc.tile_pool(name="work", bufs=3) as wk, \
             tc.tile_pool(name="small", bufs=4) as sm, \
             tc.tile_pool(name="psum", bufs=4, space="PSUM") as ps, \
             tc.tile_pool(name="psum2", bufs=2, space="PSUM") as ps2:

            Wl_sb = cp.tile([F, F], f32, tag="wl")
            Wr_sb = cp.tile([F, F], f32, tag="wr")
            att_sb = cp.tile([P, F], f32, tag="att")
            bias_sb = cp.tile([P, F], f32, tag="bias")
            idx_sb = cp.tile([P, IW], mybir.dt.int16, tag="idx")
            msk_sb = cp.tile([P, ADW], f32, tag="msk")
            nc.sync.dma_start(out=Wl_sb[:], in_=t_Wl[:])
            nc.sync.dma_start(out=Wr_sb[:], in_=t_Wr[:])
            nc.sync.dma_start(out=att_sb[:], in_=t_att[:])
            nc.sync.dma_start(out=bias_sb[:], in_=t_bias[:])
            nc.sync.dma_start(out=idx_sb[:], in_=t_idx[:])
            nc.sync.dma_start(out=msk_sb[:], in_=t_msk[:])

            # ---- phase 1: xl table = x @ Wl ----
            for j in range(n_xtile):
                w = min(XT_TILE, N - j * XT_TILE)
                nt = (w + P - 1) // P
                xt = p1.tile([F, XT_TILE], f32, tag="xt")
                nc.scalar.dma_start(out=xt[:, :w],
                                  in_=t_xT[:, j * XT_TILE:j * XT_TILE + w])
                ob = p1.tile([P, nt, F], f32, tag="ob")
                for t in range(nt):
                    wt = min(P, w - t * P)
                    mm = ps.tile([P, F], f32, tag="mm", space="PSUM")
                    nc.tensor.matmul(out=mm[:wt, :],
                                     lhsT=xt[:, t * P:t * P + wt],
                                     rhs=Wl_sb[:], start=True, stop=True)
                    nc.scalar.copy(out=ob[:wt, t, :], in_=mm[:wt, :])
                if w == XT_TILE:
                    dst = t_xl[j * XT_TILE:j * XT_TILE + w, :]
                    nc.sync.dma_start(
                        out=dst.rearrange("(t p) f -> p t f", p=P),
                        in_=ob[:, :nt, :])
                else:
                    for t in range(nt):
                        wt = min(P, w - t * P)
                        nc.sync.dma_start(
                            out=t_xl[j * XT_TILE + t * P:
                                     j * XT_TILE + t * P + wt, :],
                            in_=ob[:wt, t, :])

            # ---- phase 2: per destination batch ----
            for b in sorted(range(NB), key=lambda bb: -int(D_b[bb])):
                D = int(D_b[b])
                ao = int(a_off[b])
                go = int(g_off[b])

                xpt = sm.tile([F, P], f32, tag="xpt")
                nc.sync.dma_start(out=xpt[:], in_=t_xpT[:, b * P:(b + 1) * P])
                xr_ps = ps2.tile([P, F], f32, tag="xr_ps", space="PSUM")
                nc.tensor.matmul(out=xr_ps[:], lhsT=xpt[:], rhs=Wr_sb[:],
                                 start=True, stop=True)
                xr_sb = sm.tile([P, F], f32, tag="xr")
                nc.vector.tensor_copy(out=xr_sb[:], in_=xr_ps[:])

                xlg = wk.tile([P, DMAX, F], f32, tag="xlg")
                # slot 0: self-loop xl row via PE
                xs_ps = ps2.tile([P, F], f32, tag="xs_ps", space="PSUM")
                nc.tensor.matmul(out=xs_ps[:], lhsT=xpt[:], rhs=Wl_sb[:],
                                 start=True, stop=True)
                nc.scalar.copy(out=xlg[:, 0, :], in_=xs_ps[:])
                # slots 1..D-1: gathered
                nc.gpsimd.dma_gather(
                    out_ap=xlg[:, 1:D, :], in_ap=t_xl[HALF:, :],
                    idxs_ap=idx_sb[:, 8 * go:8 * (go + D - 1)],
                    num_idxs=P * (D - 1), num_idxs_reg=P * (D - 1),
                    elem_size=F, single_packet=False)

                # z = xlg + xr ; m = prelu(z); t = m * att (in place)
                z = wk.tile([P, DMAX, F], f32, tag="z")
                nc.vector.tensor_tensor(
                    out=z[:, :D, :], in0=xlg[:, :D, :],
                    in1=xr_sb[:].unsqueeze(1).to_broadcast([P, D, F]),
                    op=Alu.add)
                nc.scalar.activation(out=z[:, :D, :], in_=z[:, :D, :],
                                     func=Act.Prelu, alpha=NEG_SLOPE)
                nc.vector.tensor_tensor(
                    out=z[:, :D, :], in0=z[:, :D, :],
                    in1=att_sb[:].unsqueeze(1).to_broadcast([P, D, F]),
                    op=Alu.mult)

                # lg[p,d,h] = sum_c t[p,d,h,c]  (d-major, contiguous write)
                lg = sm.tile([P, DMAX, H], f32, tag="lg")
                nc.vector.tensor_reduce(
                    out=lg[:, :D, :],
                    in_=z[:, :D, :].rearrange("p d (h c) -> p d h c", h=H),
                    axis=mybir.AxisListType.X, op=Alu.add)
                nc.scalar.activation(out=lg[:, :D, :], in_=lg[:, :D, :],
                                     func=Act.Exp)
                nc.vector.tensor_tensor(
                    out=lg[:, :D, :], in0=lg[:, :D, :],
                    in1=msk_sb[:, ao:ao + D].unsqueeze(2).to_broadcast([P, D, H]),
                    op=Alu.mult)
                # den[p,h] = sum_d ex[p,d,h] (strided view, small)
                den = sm.tile([P, H], f32, tag="den")
                nc.vector.tensor_reduce(
                    out=den[:], in_=lg[:, :D, :].transpose([0, 2, 1]),
                    axis=mybir.AxisListType.X, op=Alu.add)
                rcp = sm.tile([P, H], f32, tag="rcp")
                nc.vector.reciprocal(out=rcp[:], in_=den[:])
                # alpha = ex * rcp (in place over lg)
                nc.vector.tensor_tensor(
                    out=lg[:, :D, :], in0=lg[:, :D, :],
                    in1=rcp[:].unsqueeze(1).to_broadcast([P, D, H]),
                    op=Alu.mult)
                nc.sync.dma_start(
                    out=t_alpha[:, 4 * ao:4 * (ao + D)].rearrange(
                        "p (d h) -> p d h", h=H),
                    in_=lg[:, :D, :])

                # wm (f-major into z bytes) = alpha * xlg
                z_flat = z[:].rearrange("p d f -> p (d f)")[:, :D * F]
                wm_wr = z_flat.rearrange("p (h c d) -> p h c d", h=H, c=C)
                xlg_rd = xlg[:, :D, :].rearrange(
                    "p d (h c) -> p d h c", h=H).transpose([0, 2, 3, 1])
                a_rd = lg[:, :D, :].transpose([0, 2, 1]).unsqueeze(2) \
                    .to_broadcast([P, H, C, D])
                nc.vector.tensor_tensor(out=wm_wr, in0=a_rd, in1=xlg_rd,
                                        op=Alu.mult)
                outv = sm.tile([P, F], f32, tag="outv")
                nc.vector.tensor_reduce(
                    out=outv[:],
                    in_=z_flat.rearrange("p (f d) -> p f d", d=D),
                    axis=mybir.AxisListType.X, op=Alu.add)
                nc.vector.tensor_tensor(out=outv[:], in0=outv[:],
                                        in1=bias_sb[:], op=Alu.add)
                nc.scalar.activation(out=outv[:], in_=outv[:],
                                     func=Act.Relu)
                nc.sync.dma_start(out=t_out[:, b * F:(b + 1) * F], in_=outv[:])

    nc.compile()
    return nc


LAST_EXEC_NS = None


def kernel(x, edge_index, Wl, Wr, att, bias):
    global LAST_EXEC_NS
    from concourse import bass_utils

    x = np.asarray(x, dtype=np.float32)
    ei = np.asarray(edge_index)
    Wl = np.asarray(Wl, dtype=np.float32)
    Wr = np.asarray(Wr, dtype=np.float32)
    att = np.asarray(att, dtype=np.float32)
    bias = np.asarray(bias, dtype=np.float32)

    src_t = np.concatenate([ei[0], np.arange(N)]).astype(np.int64)
    dst_t = np.concatenate([ei[1], np.arange(N)]).astype(np.int64)
    pp = _host_prep(src_t, dst_t)
    D_b, a_off = pp["D_b"], pp["a_off"]

    nc = _build_program(D_b, a_off, pp["g_off"], pp["ADW"], pp["GW"], pp["IW"])

    xT = np.ascontiguousarray(x.T)
    attr = np.tile(att.reshape(1, H * C), (P, 1)).astype(np.float32)
    biasr = np.tile(bias.reshape(1, F), (P, 1)).astype(np.float32)

    in_maps = []
    for c in range(NC):
        nodes = pp["nodes_cb"][c]
        xp = np.zeros((NLOC_PAD, F), dtype=np.float32)
        valid = nodes >= 0
        xp[valid] = x[nodes[valid]]
        in_maps.append({
            "xT": xT, "xpT": np.ascontiguousarray(xp.T),
            "Wl": Wl, "Wr": Wr, "attr": attr, "biasr": biasr,
            "idxp": pp["idx_pack"][c], "mskp": pp["msk_pack"][c],
        })

    res = bass_utils.run_bass_kernel_spmd(nc, in_maps, core_ids=list(range(NC)))
    LAST_EXEC_NS = res.exec_time_ns

    out_full = np.zeros((N, F), dtype=np.float32)
    Et = src_t.shape[0]
    alpha_full = np.zeros((Et, H), dtype=np.float32)
    abufs = np.stack([res.results[c]["alphab"] for c in range(NC)])
    for c in range(NC):
        ob = res.results[c]["outb"]
        nodes = pp["nodes_cb"][c]
        valid = nodes >= 0
        rows = ob.reshape(P, NB, F).transpose(1, 0, 2).reshape(NLOC_PAD, F)
        out_full[nodes[valid]] = rows[valid]
    e_b, e_d, e_part, e_core = pp["e_b"], pp["e_d"], pp["e_part"], pp["e_core"]
    # alpha layout: [p, d, h] per batch -> col = 4*a_off[b] + d*H + h
    for h in range(H):
        col = 4 * a_off[e_b] + e_d * H + h
        alpha_full[:, h] = abufs[e_core, e_part, col]

    return out_full, alpha_full


# revision 12
# speedup vs baseline: 1.1108x; 1.1108x over previous
"""GATv2 block (N=50000, F=128, H=4, C=32, E=800000) on 8 Trainium2 NeuronCores.

Strategy (dst-node sharding, degree-rank balanced):
  - Nodes assigned to cores by degree rank (rank r -> core r % 8): each core
    owns 6250 destinations and every core's batch b has the same max degree
    -> one shared SPMD program, balanced edge counts.
  - Per core, destinations are processed in 49 batches of 128 nodes
    (partition = node). Slot 0 of every node is its self-loop (computed by a
    PE matmul, not gathered); slots 1..D-1 are neighbor edges whose xl rows
    are fetched with one batched SWDGE dma_gather per batch (int16 indices,
    table base at row 32768 so signed indices span all 50000 rows).
  - Softmax + aggregation are free-dim vector ops (partition = dst node):
      z = xl[src] + xr[dst]; m = prelu(z, 0.2); lg[d,h] = sum_c m*att
      ex = exp(lg) * mask; den = sum_d ex; alpha = ex/den
      out = relu(sum_d alpha * xl[src] + bias)
  - xl = x @ Wl is computed on-device per core into a DRAM table for the
    gathers; xr and xl_self come from batch-ordered x (xpT input).
"""
import sys

sys.path.insert(0, "/opt/trn_rl_repo")

import numpy as np

N, F, H, C = 50000, 128, 4, 32
NC, P, HALF = 8, 128, 32768
NEG_SLOPE = 0.2
NLOC = N // NC            # 6250
NB = (NLOC + P - 1) // P  # 49
NLOC_PAD = NB * P         # 6272
XT_TILE = 2048


def _host_prep(src_t, dst_t):
    """Batch structure with self-loop at slot 0 of each node."""
    deg = np.bincount(dst_t, minlength=N)          # includes self loop
    rank = np.argsort(deg, kind="stable")
    node_core = np.empty(N, dtype=np.int64)
    node_slot = np.empty(N, dtype=np.int64)
    node_core[rank] = np.arange(N) % NC
    node_slot[rank] = np.arange(N) // NC

    eorder = np.argsort(dst_t, kind="stable")
    starts = np.zeros(N + 1, dtype=np.int64)
    starts[1:] = np.cumsum(deg)

    node_batch = node_slot // P
    D_b = np.zeros(NB, dtype=np.int64)
    np.maximum.at(D_b, node_batch, deg)
    D_b = np.maximum(D_b, 2)                       # >=1 gathered slot
    a_off = np.concatenate([[0], np.cumsum(D_b)])[:-1]
    ADW = int(D_b.sum())
    GW = int((D_b - 1).sum())                      # gathered slots per node
    g_off = np.concatenate([[0], np.cumsum(D_b - 1)])[:-1]
    IW = 8 * GW

    Et = src_t.shape[0]
    idx_pack = np.zeros((NC, P, IW), dtype=np.int16)
    msk_pack = np.zeros((NC, P, ADW), dtype=np.float32)
    nodes_cb = np.full((NC, NLOC_PAD), -1, dtype=np.int64)
    e_part = np.zeros(Et, dtype=np.int64)
    e_d = np.zeros(Et, dtype=np.int64)
    e_b = node_batch[dst_t]
    e_core = node_core[dst_t]
    selfloop_eid = Et - N + np.arange(N)           # self loop of node n

    for c in range(NC):
        has = node_core == c
        slot_nodes = np.full(NLOC_PAD, -1, dtype=np.int64)
        slot_nodes[node_slot[has]] = np.nonzero(has)[0]
        for b in range(NB):
            D = int(D_b[b])
            nb = slot_nodes[b * P:(b + 1) * P].copy()
            degs = np.where(nb >= 0, deg[np.maximum(nb, 0)], 0)
            # partition-127 node must have a tail-safe last gather slot:
            # pad (deg < D) or a hi (>= HALF) non-self edge to place last.
            reorder_hi_last = False
            if degs[P - 1] >= D:
                cand = np.nonzero(degs < D)[0]
                if len(cand):
                    j = int(cand[0])
                    nb[[j, P - 1]] = nb[[P - 1, j]]
                else:
                    ok = -1
                    for j in range(P - 1, -1, -1):
                        n = nb[j]
                        if n < 0:
                            continue
                        ee = eorder[starts[n]:starts[n + 1]]
                        if (src_t[ee[ee != selfloop_eid[n]]] >= HALF).any():
                            ok = j
                            break
                    assert ok >= 0, "no hi-src edge in batch"
                    nb[[ok, P - 1]] = nb[[P - 1, ok]]
                    reorder_hi_last = True
            nodes_cb[c, b * P:(b + 1) * P] = nb

            gi = np.full((P, D - 1), HALF, dtype=np.int64)  # pads: row HALF
            for j in range(P):
                n = nb[j]
                if n < 0:
                    continue
                ee = eorder[starts[n]:starts[n + 1]]
                se = selfloop_eid[n]
                ee_n = ee[ee != se]                  # non-self edges
                if j == P - 1 and reorder_hi_last:
                    ee_n = ee_n[np.argsort(src_t[ee_n] >= HALF, kind="stable")]
                # slot 0 = self loop
                e_part[se] = j
                e_d[se] = 0
                msk_pack[c, j, a_off[b]] = 1.0
                k = len(ee_n)
                gi[j, :k] = src_t[ee_n]
                e_part[ee_n] = j
                e_d[ee_n] = 1 + np.arange(k)
                msk_pack[c, j, a_off[b] + 1:a_off[b] + 1 + k] = 1.0
            assert gi[P - 1, D - 2] >= HALF, (c, b)
            lst = gi.T.reshape(P * (D - 1))
            w = (lst - HALF).astype(np.int16).reshape(-1, 16).T
            idx_pack[c, :, 8 * g_off[b]:8 * (g_off[b] + D - 1)] = np.tile(w, (8, 1))

    return dict(D_b=D_b, a_off=a_off, g_off=g_off, ADW=ADW, GW=GW, IW=IW,
                idx_pack=idx_pack, msk_pack=msk_pack, nodes_cb=nodes_cb,
                e_core=e_core, e_part=e_part, e_d=e_d, e_b=e_b)


def _build_program(D_b, a_off, g_off, ADW, GW, IW):
    import concourse.bacc as bacc
    import concourse.mybir as mybir
    import concourse.tile as tile

    f32 = mybir.dt.float32
    Alu = mybir.AluOpType
    Act = mybir.ActivationFunctionType

    nc = bacc.Bacc("TRN2", target_bir_lowering=False)

    t_xT = nc.dram_tensor("xT", [F, N], f32, kind="ExternalInput")
    t_xpT = nc.dram_tensor("xpT", [F, NLOC_PAD], f32, kind="ExternalInput")
    t_Wl = nc.dram_tensor("Wl", [F, F], f32, kind="ExternalInput")
    t_Wr = nc.dram_tensor("Wr", [F, F], f32, kind="ExternalInput")
    t_att = nc.dram_tensor("attr", [P, F], f32, kind="ExternalInput")
    t_bias = nc.dram_tensor("biasr", [P, F], f32, kind="ExternalInput")
    t_idx = nc.dram_tensor("idxp", [P, IW], mybir.dt.int16, kind="ExternalInput")
    t_msk = nc.dram_tensor("mskp", [P, ADW], f32, kind="ExternalInput")
    t_out = nc.dram_tensor("outb", [P, NB * F], f32, kind="ExternalOutput")
    t_alpha = nc.dram_tensor("alphab", [P, 4 * ADW], f32, kind="ExternalOutput")
    t_xl = nc.dram_tensor("xl_table", [N, F], f32)

    DMAX = int(max(D_b))
    n_xtile = (N + XT_TILE - 1) // XT_TILE

    with tile.TileContext(nc) as tc:
        with tc.tile_pool(name="const", bufs=1) as cp, \
             tc.tile_pool(name="ph1", bufs=2) as p1, \
             tc.tile_pool(name="work", bufs=3) as wk, \
             tc.tile_pool(name="small", bufs=4) as sm, \
             tc.tile_pool(name="psum", bufs=4, space="PSUM") as ps, \
             tc.tile_pool(name="psum2", bufs=2, space="PSUM") as ps2:

            Wl_sb = cp.tile([F, F], f32, tag="wl")
            Wr_sb = cp.tile([F, F], f32, tag="wr")
            att_sb = cp.tile([P, F], f32, tag="att")
            bias_sb = cp.tile([P, F], f32, tag="bias")
            idx_sb = cp.tile([P, IW], mybir.dt.int16, tag="idx")
            msk_sb = cp.tile([P, ADW], f32, tag="msk")
            nc.sync.dma_start(out=Wl_sb[:], in_=t_Wl[:])
            nc.sync.dma_start(out=Wr_sb[:], in_=t_Wr[:])
            nc.sync.dma_start(out=att_sb[:], in_=t_att[:])
            nc.sync.dma_start(out=bias_sb[:], in_=t_bias[:])
            nc.sync.dma_start(out=idx_sb[:], in_=t_idx[:])
            nc.sync.dma_start(out=msk_sb[:], in_=t_msk[:])

            # ---- phase 1: xl table = x @ Wl ----
            for j in range(n_xtile):
                w = min(XT_TILE, N - j * XT_TILE)
                nt = (w + P - 1) // P
                xt = p1.tile([F, XT_TILE], f32, tag="xt")
                nc.scalar.dma_start(out=xt[:, :w],
                                  in_=t_xT[:, j * XT_TILE:j * XT_TILE + w])
                ob = p1.tile([P, nt, F], f32, tag="ob")
                for t in range(nt):
                    wt = min(P, w - t * P)
                    mm = ps.tile([P, F], f32, tag="mm", space="PSUM")
                    nc.tensor.matmul(out=mm[:wt, :],
                                     lhsT=xt[:, t * P:t * P + wt],
                                     rhs=Wl_sb[:], start=True, stop=True)
                    nc.scalar.copy(out=ob[:wt, t, :], in_=mm[:wt, :])
                if w == XT_TILE:
                    dst = t_xl[j * XT_TILE:j * XT_TILE + w, :]
                    nc.sync.dma_start(
                        out=dst.rearrange("(t p) f -> p t f", p=P),
                        in_=ob[:, :nt, :])
                else:
                    for t in range(nt):
                        wt = min(P, w - t * P)
                        nc.sync.dma_start(
                            out=t_xl[j * XT_TILE + t * P:
                                     j * XT_TILE + t * P + wt, :],
                            in_=ob[:wt, t, :])

            # ---- phase 2: per destination batch ----
            for b in sorted(range(NB), key=lambda bb: -int(D_b[bb])):
                D = int(D_b[b])
                ao = int(a_off[b])
                go = int(g_off[b])

                xpt = sm.tile([F, P], f32, tag="xpt")
                nc.sync.dma_start(out=xpt[:], in_=t_xpT[:, b * P:(b + 1) * P])
                xr_ps = ps2.tile([P, F], f32, tag="xr_ps", space="PSUM")
                nc.tensor.matmul(out=xr_ps[:], lhsT=xpt[:], rhs=Wr_sb[:],
                                 start=True, stop=True)
                xr_sb = sm.tile([P, F], f32, tag="xr")
                nc.vector.tensor_copy(out=xr_sb[:], in_=xr_ps[:])

                xlg = wk.tile([P, DMAX, F], f32, tag="xlg")
                # slot 0: self-loop xl row via PE
                xs_ps = ps2.tile([P, F], f32, tag="xs_ps", space="PSUM")
                nc.tensor.matmul(out=xs_ps[:], lhsT=xpt[:], rhs=Wl_sb[:],
                                 start=True, stop=True)
                nc.scalar.copy(out=xlg[:, 0, :], in_=xs_ps[:])
                # slots 1..D-1: gathered
                nc.gpsimd.dma_gather(
                    out_ap=xlg[:, 1:D, :], in_ap=t_xl[HALF:, :],
                    idxs_ap=idx_sb[:, 8 * go:8 * (go + D - 1)],
                    num_idxs=P * (D - 1), num_idxs_reg=P * (D - 1),
                    elem_size=F, single_packet=False)

                # z = xlg + xr ; m = prelu(z); t = m * att (in place)
                z = wk.tile([P, DMAX, F], f32, tag="z")
                nc.vector.tensor_tensor(
                    out=z[:, :D, :], in0=xlg[:, :D, :],
                    in1=xr_sb[:].unsqueeze(1).to_broadcast([P, D, F]),
                    op=Alu.add)
                nc.vector.scalar_tensor_tensor(
                    out=z[:, :D, :], in0=z[:, :D, :], scalar=NEG_SLOPE,
                    in1=z[:, :D, :], op0=Alu.mult, op1=Alu.max)
                nc.vector.tensor_tensor(
                    out=z[:, :D, :], in0=z[:, :D, :],
                    in1=att_sb[:].unsqueeze(1).to_broadcast([P, D, F]),
                    op=Alu.mult)

                # lg[p,d,h] = sum_c t[p,d,h,c]  (d-major, contiguous write)
                lg = sm.tile([P, DMAX, H], f32, tag="lg")
                nc.vector.tensor_reduce(
                    out=lg[:, :D, :],
                    in_=z[:, :D, :].rearrange("p d (h c) -> p d h c", h=H),
                    axis=mybir.AxisListType.X, op=Alu.add)
                nc.scalar.activation(out=lg[:, :D, :], in_=lg[:, :D, :],
                                     func=Act.Exp)
                nc.vector.tensor_tensor(
                    out=lg[:, :D, :], in0=lg[:, :D, :],
                    in1=msk_sb[:, ao:ao + D].unsqueeze(2).to_broadcast([P, D, H]),
                    op=Alu.mult)
                # den[p,h] = sum_d ex[p,d,h] (strided view, small)
                den = sm.tile([P, H], f32, tag="den")
                nc.vector.tensor_reduce(
                    out=den[:], in_=lg[:, :D, :].transpose([0, 2, 1]),
                    axis=mybir.AxisListType.X, op=Alu.add)
                rcp = sm.tile([P, H], f32, tag="rcp")
                nc.vector.reciprocal(out=rcp[:], in_=den[:])
                # alpha = ex * rcp (in place over lg)
                nc.vector.tensor_tensor(
                    out=lg[:, :D, :], in0=lg[:, :D, :],
                    in1=rcp[:].unsqueeze(1).to_broadcast([P, D, H]),
                    op=Alu.mult)
                nc.sync.dma_start(
                    out=t_alpha[:, 4 * ao:4 * (ao + D)].rearrange(
                        "p (d h) -> p d h", h=H),
                    in_=lg[:, :D, :])

                # wm (f-major into z bytes) = alpha * xlg
                z_flat = z[:].rearrange("p d f -> p (d f)")[:, :D * F]
                wm_wr = z_flat.rearrange("p (h c d) -> p h c d", h=H, c=C)
                xlg_rd = xlg[:, :D, :].rearrange(
                    "p d (h c) -> p d h c", h=H).transpose([0, 2, 3, 1])
                a_rd = lg[:, :D, :].transpose([0, 2, 1]).unsqueeze(2) \
                    .to_broadcast([P, H, C, D])
                nc.vector.tensor_tensor(out=wm_wr, in0=a_rd, in1=xlg_rd,
                                        op=Alu.mult)
                outv = sm.tile([P, F], f32, tag="outv")
                nc.vector.tensor_reduce(
                    out=outv[:],
                    in_=z_flat.rearrange("p (f d) -> p f d", d=D),
                    axis=mybir.AxisListType.X, op=Alu.add)
                nc.vector.tensor_tensor(out=outv[:], in0=outv[:],
                                        in1=bias_sb[:], op=Alu.add)
                nc.scalar.activation(out=outv[:], in_=outv[:],
                                     func=Act.Relu)
                nc.sync.dma_start(out=t_out[:, b * F:(b + 1) * F], in_=outv[:])

    nc.compile()
    return nc


LAST_EXEC_NS = None


def kernel(x, edge_index, Wl, Wr, att, bias):
    global LAST_EXEC_NS
    from concourse import bass_utils

    x = np.asarray(x, dtype=np.float32)
    ei = np.asarray(edge_index)
    Wl = np.asarray(Wl, dtype=np.float32)
    Wr = np.asarray(Wr, dtype=np.float32)
    att = np.asarray(att, dtype=np.float32)
    bias = np.asarray(bias, dtype=np.float32)

    src_t = np.concatenate([ei[0], np.arange(N)]).astype(np.int64)
    dst_t = np.concatenate([ei[1], np.arange(N)]).astype(np.int64)
    pp = _host_prep(src_t, dst_t)
    D_b, a_off = pp["D_b"], pp["a_off"]

    nc = _build_program(D_b, a_off, pp["g_off"], pp["ADW"], pp["GW"], pp["IW"])

    xT = np.ascontiguousarray(x.T)
    attr = np.tile(att.reshape(1, H * C), (P, 1)).astype(np.float32)
    biasr = np.tile(bias.reshape(1, F), (P, 1)).astype(np.float32)

    in_maps = []
    for c in range(NC):
        nodes = pp["nodes_cb"][c]
        xp = np.zeros((NLOC_PAD, F), dtype=np.float32)
        valid = nodes >= 0
        xp[valid] = x[nodes[valid]]
        in_maps.append({
            "xT": xT, "xpT": np.ascontiguousarray(xp.T),
            "Wl": Wl, "Wr": Wr, "attr": attr, "biasr": biasr,
            "idxp": pp["idx_pack"][c], "mskp": pp["msk_pack"][c],
        })

    res = bass_utils.run_bass_kernel_spmd(nc, in_maps, core_ids=list(range(NC)))
    LAST_EXEC_NS = res.exec_time_ns

    out_full = np.zeros((N, F), dtype=np.float32)
    Et = src_t.shape[0]
    alpha_full = np.zeros((Et, H), dtype=np.float32)
    abufs = np.stack([res.results[c]["alphab"] for c in range(NC)])
    for c in range(NC):
        ob = res.results[c]["outb"]
        nodes = pp["nodes_cb"][c]
        valid = nodes >= 0
        rows = ob.reshape(P, NB, F).transpose(1, 0, 2).reshape(NLOC_PAD, F)
        out_full[nodes[valid]] = rows[valid]
    e_b, e_d, e_part, e_core = pp["e_b"], pp["e_d"], pp["e_part"], pp["e_core"]
    # alpha layout: [p, d, h] per batch -> col = 4*a_off[b] + d*H + h
    for h in range(H):
        col = 4 * a_off[e_b] + e_d * H + h
        alpha_full[:, h] = abufs[e_core, e_part, col]

    return out_full, alpha_full


# revision 13
# speedup vs baseline: 1.3693x; 1.2327x over previous
"""GATv2 block (N=50000, F=128, H=4, C=32, E=800000) on 8 Trainium2 NeuronCores.

Strategy (dst-node sharding, degree-rank balanced):
  - Nodes assigned to cores by degree rank (rank r -> core r % 8): each core
    owns 6250 destinations and every core's batch b has the same max degree
    -> one shared SPMD program, balanced edge counts.
  - Per core, destinations are processed in 49 batches of 128 nodes
    (partition = node). Slot 0 of every node is its self-loop (computed by a
    PE matmul, not gathered); slots 1..D-1 are neighbor edges whose xl rows
    are fetched with one batched SWDGE dma_gather per batch (int16 indices,
    table base at row 32768 so signed indices span all 50000 rows).
  - Softmax + aggregation are free-dim vector ops (partition = dst node):
      z = xl[src] + xr[dst]; m = prelu(z, 0.2); lg[d,h] = sum_c m*att
      ex = exp(lg) * mask; den = sum_d ex; alpha = ex/den
      out = relu(sum_d alpha * xl[src] + bias)
  - xl = x @ Wl is computed on-device per core into a DRAM table for the
    gathers; xr and xl_self come from batch-ordered x (xpT input).
"""
import sys

sys.path.insert(0, "/opt/trn_rl_repo")

import numpy as np

N, F, H, C = 50000, 128, 4, 32
NC, P, HALF = 8, 128, 32768
NEG_SLOPE = 0.2
NLOC = N // NC            # 6250
NB = (NLOC + P - 1) // P  # 49
NLOC_PAD = NB * P         # 6272
XT_TILE = 2048


def _host_prep(src_t, dst_t):
    """Batch structure with self-loop at slot 0 of each node."""
    deg = np.bincount(dst_t, minlength=N)          # includes self loop
    rank = np.argsort(deg, kind="stable")
    node_core = np.empty(N, dtype=np.int64)
    node_slot = np.empty(N, dtype=np.int64)
    node_core[rank] = np.arange(N) % NC
    node_slot[rank] = np.arange(N) // NC

    eorder = np.argsort(dst_t, kind="stable")
    starts = np.zeros(N + 1, dtype=np.int64)
    starts[1:] = np.cumsum(deg)

    node_batch = node_slot // P
    D_b = np.zeros(NB, dtype=np.int64)
    np.maximum.at(D_b, node_batch, deg)
    D_b = np.maximum(D_b, 2)                       # >=1 gathered slot
    a_off = np.concatenate([[0], np.cumsum(D_b)])[:-1]
    ADW = int(D_b.sum())
    GW = int((D_b - 1).sum())                      # gathered slots per node
    g_off = np.concatenate([[0], np.cumsum(D_b - 1)])[:-1]
    IW = 8 * GW

    Et = src_t.shape[0]
    idx_pack = np.zeros((NC, P, IW), dtype=np.int16)
    msk_pack = np.zeros((NC, P, ADW), dtype=np.float32)
    nodes_cb = np.full((NC, NLOC_PAD), -1, dtype=np.int64)
    e_part = np.zeros(Et, dtype=np.int64)
    e_d = np.zeros(Et, dtype=np.int64)
    e_b = node_batch[dst_t]
    e_core = node_core[dst_t]
    selfloop_eid = Et - N + np.arange(N)           # self loop of node n

    for c in range(NC):
        has = node_core == c
        slot_nodes = np.full(NLOC_PAD, -1, dtype=np.int64)
        slot_nodes[node_slot[has]] = np.nonzero(has)[0]
        for b in range(NB):
            D = int(D_b[b])
            nb = slot_nodes[b * P:(b + 1) * P].copy()
            degs = np.where(nb >= 0, deg[np.maximum(nb, 0)], 0)
            # partition-127 node must have a tail-safe last gather slot:
            # pad (deg < D) or a hi (>= HALF) non-self edge to place last.
            reorder_hi_last = False
            if degs[P - 1] >= D:
                cand = np.nonzero(degs < D)[0]
                if len(cand):
                    j = int(cand[0])
                    nb[[j, P - 1]] = nb[[P - 1, j]]
                else:
                    ok = -1
                    for j in range(P - 1, -1, -1):
                        n = nb[j]
                        if n < 0:
                            continue
                        ee = eorder[starts[n]:starts[n + 1]]
                        if (src_t[ee[ee != selfloop_eid[n]]] >= HALF).any():
                            ok = j
                            break
                    assert ok >= 0, "no hi-src edge in batch"
                    nb[[ok, P - 1]] = nb[[P - 1, ok]]
                    reorder_hi_last = True
            nodes_cb[c, b * P:(b + 1) * P] = nb

            gi = np.full((P, D - 1), HALF, dtype=np.int64)  # pads: row HALF
            for j in range(P):
                n = nb[j]
                if n < 0:
                    continue
                ee = eorder[starts[n]:starts[n + 1]]
                se = selfloop_eid[n]
                ee_n = ee[ee != se]                  # non-self edges
                if j == P - 1 and reorder_hi_last:
                    ee_n = ee_n[np.argsort(src_t[ee_n] >= HALF, kind="stable")]
                # slot 0 = self loop
                e_part[se] = j
                e_d[se] = 0
                msk_pack[c, j, a_off[b]] = 1.0
                k = len(ee_n)
                gi[j, :k] = src_t[ee_n]
                e_part[ee_n] = j
                e_d[ee_n] = 1 + np.arange(k)
                msk_pack[c, j, a_off[b] + 1:a_off[b] + 1 + k] = 1.0
            assert gi[P - 1, D - 2] >= HALF, (c, b)
            lst = gi.T.reshape(P * (D - 1))
            w = (lst - HALF).astype(np.int16).reshape(-1, 16).T
            idx_pack[c, :, 8 * g_off[b]:8 * (g_off[b] + D - 1)] = np.tile(w, (8, 1))

    return dict(D_b=D_b, a_off=a_off, g_off=g_off, ADW=ADW, GW=GW, IW=IW,
                idx_pack=idx_pack, msk_pack=msk_pack, nodes_cb=nodes_cb,
                e_core=e_core, e_part=e_part, e_d=e_d, e_b=e_b)


def _build_program(D_b, a_off, g_off, ADW, GW, IW):
    import concourse.bacc as bacc
    import concourse.mybir as mybir
    import concourse.tile as tile
    from concourse.masks import make_identity

    f32 = mybir.dt.float32
    Alu = mybir.AluOpType
    Act = mybir.ActivationFunctionType

    nc = bacc.Bacc("TRN2", target_bir_lowering=False)

    t_x = nc.dram_tensor("xrow", [N, F], f32, kind="ExternalInput")
    t_xpT = nc.dram_tensor("xpT", [F, NLOC_PAD], f32, kind="ExternalInput")
    t_Wl = nc.dram_tensor("Wl", [F, F], f32, kind="ExternalInput")
    t_Wr = nc.dram_tensor("Wr", [F, F], f32, kind="ExternalInput")
    t_att = nc.dram_tensor("attr", [P, F], f32, kind="ExternalInput")
    t_bias = nc.dram_tensor("biasr", [P, F], f32, kind="ExternalInput")
    t_idx = nc.dram_tensor("idxp", [P, IW], mybir.dt.int16, kind="ExternalInput")
    t_msk = nc.dram_tensor("mskp", [P, ADW], f32, kind="ExternalInput")
    t_out = nc.dram_tensor("outb", [P, NB * F], f32, kind="ExternalOutput")
    t_alpha = nc.dram_tensor("alphab", [P, 4 * ADW], f32, kind="ExternalOutput")

    DMAX = int(max(D_b))
    n_xtile = (N + XT_TILE - 1) // XT_TILE

    with tile.TileContext(nc) as tc:
        with tc.tile_pool(name="const", bufs=1) as cp, \
             tc.tile_pool(name="work", bufs=3) as wk, \
             tc.tile_pool(name="small", bufs=4) as sm, \
             tc.tile_pool(name="psum", bufs=2, space="PSUM") as ps, \
             tc.tile_pool(name="psum2", bufs=2, space="PSUM") as ps2:

            Wl_sb = cp.tile([F, F], f32, tag="wl")
            Wr_sb = cp.tile([F, F], f32, tag="wr")
            att_sb = cp.tile([P, F], f32, tag="att")
            bias_sb = cp.tile([P, F], f32, tag="bias")
            idx_sb = cp.tile([P, IW], mybir.dt.int16, tag="idx")
            msk_sb = cp.tile([P, ADW], f32, tag="msk")
            ident = cp.tile([P, P], f32, tag="ident")
            make_identity(nc, ident[:])
            nc.sync.dma_start(out=Wl_sb[:], in_=t_Wl[:])
            nc.sync.dma_start(out=Wr_sb[:], in_=t_Wr[:])
            nc.sync.dma_start(out=att_sb[:], in_=t_att[:])
            nc.sync.dma_start(out=bias_sb[:], in_=t_bias[:])
            nc.sync.dma_start(out=idx_sb[:], in_=t_idx[:])
            nc.sync.dma_start(out=msk_sb[:], in_=t_msk[:])

            # ---- phase 2: per destination batch ----
            for b in sorted(range(NB), key=lambda bb: -int(D_b[bb])):
                D = int(D_b[b])
                ao = int(a_off[b])
                go = int(g_off[b])

                xpt = sm.tile([F, P], f32, tag="xpt")
                nc.sync.dma_start(out=xpt[:], in_=t_xpT[:, b * P:(b + 1) * P])
                xr_ps = ps2.tile([P, F], f32, tag="xr_ps", space="PSUM")
                nc.tensor.matmul(out=xr_ps[:], lhsT=xpt[:], rhs=Wr_sb[:],
                                 start=True, stop=True)
                xr_sb = sm.tile([P, F], f32, tag="xr")
                nc.vector.tensor_copy(out=xr_sb[:], in_=xr_ps[:])

                xlg = wk.tile([P, DMAX, F], f32, tag="xlg")
                # slot 0: self-loop xl row via PE
                xs_ps = ps2.tile([P, F], f32, tag="xs_ps", space="PSUM")
                nc.tensor.matmul(out=xs_ps[:], lhsT=xpt[:], rhs=Wl_sb[:],
                                 start=True, stop=True)
                nc.scalar.copy(out=xlg[:, 0, :], in_=xs_ps[:])
                # slots 1..D-1: gather raw x rows, project per 4-slot chunk
                xg = wk.tile([P, DMAX, F], f32, tag="xg")
                nc.gpsimd.dma_gather(
                    out_ap=xg[:, 1:D, :], in_ap=t_x[HALF:, :],
                    idxs_ap=idx_sb[:, 8 * go:8 * (go + D - 1)],
                    num_idxs=P * (D - 1), num_idxs_reg=P * (D - 1),
                    elem_size=F, single_packet=False)
                d0 = 1
                while d0 < D:
                    dn = min(4, D - d0)
                    trp = ps.tile([P, 4, F], f32, tag="trp", space="PSUM")
                    for t in range(dn):
                        nc.tensor.transpose(out=trp[:, t, :],
                                            in_=xg[:, d0 + t, :],
                                            identity=ident[:])
                    xgt = sm.tile([P, 4, F], f32, tag="xgt")
                    nc.scalar.copy(out=xgt[:, :dn, :], in_=trp[:, :dn, :])
                    zp = ps.tile([P, 4, F], f32, tag="zp", space="PSUM")
                    for t in range(dn):
                        nc.tensor.matmul(out=zp[:, t, :], lhsT=xgt[:, t, :],
                                         rhs=Wl_sb[:], start=True, stop=True)
                    nc.scalar.copy(out=xlg[:, d0:d0 + dn, :], in_=zp[:, :dn, :])
                    d0 += dn

                # z = xlg + xr ; m = prelu(z); t = m * att (in place)
                z = wk.tile([P, DMAX, F], f32, tag="z")
                nc.vector.tensor_tensor(
                    out=z[:, :D, :], in0=xlg[:, :D, :],
                    in1=xr_sb[:].unsqueeze(1).to_broadcast([P, D, F]),
                    op=Alu.add)
                nc.vector.scalar_tensor_tensor(
                    out=z[:, :D, :], in0=z[:, :D, :], scalar=NEG_SLOPE,
                    in1=z[:, :D, :], op0=Alu.mult, op1=Alu.max)
                nc.vector.tensor_tensor(
                    out=z[:, :D, :], in0=z[:, :D, :],
                    in1=att_sb[:].unsqueeze(1).to_broadcast([P, D, F]),
                    op=Alu.mult)

                # lg[p,d,h] = sum_c t[p,d,h,c]  (d-major, contiguous write)
                lg = sm.tile([P, DMAX, H], f32, tag="lg")
                nc.vector.tensor_reduce(
                    out=lg[:, :D, :],
                    in_=z[:, :D, :].rearrange("p d (h c) -> p d h c", h=H),
                    axis=mybir.AxisListType.X, op=Alu.add)
                nc.scalar.activation(out=lg[:, :D, :], in_=lg[:, :D, :],
                                     func=Act.Exp)
                nc.vector.tensor_tensor(
                    out=lg[:, :D, :], in0=lg[:, :D, :],
                    in1=msk_sb[:, ao:ao + D].unsqueeze(2).to_broadcast([P, D, H]),
                    op=Alu.mult)
                # den[p,h] = sum_d ex[p,d,h] (strided view, small)
                den = sm.tile([P, H], f32, tag="den")
                nc.vector.tensor_reduce(
                    out=den[:], in_=lg[:, :D, :].transpose([0, 2, 1]),
                    axis=mybir.AxisListType.X, op=Alu.add)
                rcp = sm.tile([P, H], f32, tag="rcp")
                nc.vector.reciprocal(out=rcp[:], in_=den[:])
                # alpha = ex * rcp (in place over lg)
                nc.vector.tensor_tensor(
                    out=lg[:, :D, :], in0=lg[:, :D, :],
                    in1=rcp[:].unsqueeze(1).to_broadcast([P, D, H]),
                    op=Alu.mult)
                nc.sync.dma_start(
                    out=t_alpha[:, 4 * ao:4 * (ao + D)].rearrange(
                        "p (d h) -> p d h", h=H),
                    in_=lg[:, :D, :])

                # wm (f-major into z bytes) = alpha * xlg
                z_flat = z[:].rearrange("p d f -> p (d f)")[:, :D * F]
                wm_wr = z_flat.rearrange("p (h c d) -> p h c d", h=H, c=C)
                xlg_rd = xlg[:, :D, :].rearrange(
                    "p d (h c) -> p d h c", h=H).transpose([0, 2, 3, 1])
                a_rd = lg[:, :D, :].transpose([0, 2, 1]).unsqueeze(2) \
                    .to_broadcast([P, H, C, D])
                nc.vector.tensor_tensor(out=wm_wr, in0=a_rd, in1=xlg_rd,
                                        op=Alu.mult)
                outv = sm.tile([P, F], f32, tag="outv")
                nc.vector.tensor_reduce(
                    out=outv[:],
                    in_=z_flat.rearrange("p (f d) -> p f d", d=D),
                    axis=mybir.AxisListType.X, op=Alu.add)
                nc.vector.tensor_tensor(out=outv[:], in0=outv[:],
                                        in1=bias_sb[:], op=Alu.add)
                nc.scalar.activation(out=outv[:], in_=outv[:],
                                     func=Act.Relu)
                nc.sync.dma_start(out=t_out[:, b * F:(b + 1) * F], in_=outv[:])

    nc.compile()
    return nc


LAST_EXEC_NS = None


def kernel(x, edge_index, Wl, Wr, att, bias):
    global LAST_EXEC_NS
    from concourse import bass_utils

    x = np.asarray(x, dtype=np.float32)
    ei = np.asarray(edge_index)
    Wl = np.asarray(Wl, dtype=np.float32)
    Wr = np.asarray(Wr, dtype=np.float32)
    att = np.asarray(att, dtype=np.float32)
    bias = np.asarray(bias, dtype=np.float32)

    src_t = np.concatenate([ei[0], np.arange(N)]).astype(np.int64)
    dst_t = np.concatenate([ei[1], np.arange(N)]).astype(np.int64)
    pp = _host_prep(src_t, dst_t)
    D_b, a_off = pp["D_b"], pp["a_off"]

    nc = _build_program(D_b, a_off, pp["g_off"], pp["ADW"], pp["GW"], pp["IW"])

    attr = np.tile(att.reshape(1, H * C), (P, 1)).astype(np.float32)
    biasr = np.tile(bias.reshape(1, F), (P, 1)).astype(np.float32)

    in_maps = []
    for c in range(NC):
        nodes = pp["nodes_cb"][c]
        xp = np.zeros((NLOC_PAD, F), dtype=np.float32)
        valid = nodes >= 0
        xp[valid] = x[nodes[valid]]
        in_maps.append({
            "xrow": x, "xpT": np.ascontiguousarray(xp.T),
            "Wl": Wl, "Wr": Wr, "attr": attr, "biasr": biasr,
            "idxp": pp["idx_pack"][c], "mskp": pp["msk_pack"][c],
        })

    res = bass_utils.run_bass_kernel_spmd(nc, in_maps, core_ids=list(range(NC)))
    LAST_EXEC_NS = res.exec_time_ns

    out_full = np.zeros((N, F), dtype=np.float32)
    Et = src_t.shape[0]
    alpha_full = np.zeros((Et, H), dtype=np.float32)
    abufs = np.stack([res.results[c]["alphab"] for c in range(NC)])
    for c in range(NC):
        ob = res.results[c]["outb"]
        nodes = pp["nodes_cb"][c]
        valid = nodes >= 0
        rows = ob.reshape(P, NB, F).transpose(1, 0, 2).reshape(NLOC_PAD, F)
        out_full[nodes[valid]] = rows[valid]
    e_b, e_d, e_part, e_core = pp["e_b"], pp["e_d"], pp["e_part"], pp["e_core"]
    # alpha layout: [p, d, h] per batch -> col = 4*a_off[b] + d*H + h
    for h in range(H):
        col = 4 * a_off[e_b] + e_d * H + h
        alpha_full[:, h] = abufs[e_core, e_part, col]

    return out_full, alpha_full
